# revision 15
# baseline (speedup 1.0000x reference)
"""Trainium2 Bass kernel for nn_DifferentiableTopologyRegularizer.

Reference math (per batch b of 128):
  x = latent[b, ::16, :]                     # [128, 512]
  d = pairwise_euclidean(x)                  # [128, 128]
  p = sigmoid(|ct| + 0.1 - d)
  conn_sum_b = sum(p) - trace(p)
  connectivity_b = 1 - conn_sum_b / (128*127 + 1e-8)
  edges(b,k) = (d[i0,i1], d[i0,i2], d[i1,i2]) for 32 triplets
  hole_b = mean_k exp(-var(edges, ddof=1))
  loss = mean_b connectivity_b + 0.5 * mean_b hole_b

Sharding: pure data parallel, 16 batches per core across 8 cores.
Each core returns [S_conn_partial, S_hole_partial]; host averages.

Device algorithm per batch:
  G_psum = sum_c xT_c.T @ xT_c  (bf16 matmuls, f32 accum)   # Gram matrix
  sqn_col = diag(G)  via DVE multiply-with-identity + accum_out
  dsq = I * sqn (ACT per-partition scale) ; G_psum += (-0.5 ones).T @ dsq
    -> psum = G - 0.5*sqn_j
  r = Relu(-2*psum + sqn_i)  (ACT, bias=sqn_col)  = relu(||xi-xj||^2)
  d = Sqrt(r) -> bf16
  p = Sigmoid(-d + thr), accum_out -> conn_acc[:, b]
  trace via DVE multiply-with-identity + accum_out -> trace_acc[:, b]
  triplet gather: O = OneHotRows.T @ d ; edges = sum_j(O * ColMask) (DVE accum)
Tail (once): S1 = A.T @ edges, S2 = A.T @ edges^2, var = S2/2 - S1^2/6,
  hole = exp(-var); final partition reductions via ones-matmuls.
"""

import os
from contextlib import ExitStack

import numpy as np
import ml_dtypes

import concourse.bass as bass
import concourse.bacc as bacc
import concourse.mybir as mybir
import concourse.tile as tile
from concourse.masks import make_identity
from concourse.bass_utils import run_bass_kernel_spmd

F32 = mybir.dt.float32
BF16 = mybir.dt.bfloat16

N_CORES = 8
B_TOTAL = 128
B_CORE = B_TOTAL // N_CORES  # 16
TC = 128          # subsampled sequence length
D = 512
NCHUNK = D // 128  # 4
N_TRIPLETS = 32
NT = 3 * N_TRIPLETS  # 96 edges per batch
DENOM = TC * (TC - 1) + 1e-8


def _build_kernel_body(ctx, tc, xt, oh, cm, amat, ct, out):
    nc = tc.nc
    AF = mybir.ActivationFunctionType
    OP = mybir.AluOpType

    consts = ctx.enter_context(tc.tile_pool(name="consts", bufs=1))
    xpool = ctx.enter_context(tc.tile_pool(name="xpool", bufs=3))
    ohpool = ctx.enter_context(tc.tile_pool(name="ohpool", bufs=3))
    work = ctx.enter_context(tc.tile_pool(name="work", bufs=3))
    acc = ctx.enter_context(tc.tile_pool(name="acc", bufs=1))
    sqnpool = ctx.enter_context(tc.tile_pool(name="sqnpool", bufs=8))
    gpsum = ctx.enter_context(tc.tile_pool(name="gpsum", bufs=4, space="PSUM"))
    spsum = ctx.enter_context(tc.tile_pool(name="spsum", bufs=1, space="PSUM"))

    # ---- constants ----
    ident_f32 = consts.tile([128, 128], F32)
    make_identity(nc, ident_f32)
    ident_bf = consts.tile([128, 128], BF16)
    make_identity(nc, ident_bf)
    # prime the gpsimd-built identities on vector+scalar engines so the
    # cross-engine wait is absorbed here, not on per-batch instructions
    # (HW limit on sync-wait commands per instruction).
    prime_v = consts.tile([1, 1], F32)
    nc.vector.tensor_copy(out=prime_v, in_=ident_f32[0:1, 0:1])
    prime_s = consts.tile([1, 1], BF16)
    nc.scalar.copy(out=prime_s, in_=ident_bf[0:1, 0:1])
    neghalf = consts.tile([128, 128], BF16)
    nc.vector.memset(neghalf, -0.5)
    ones_col = consts.tile([128, 1], F32)
    nc.vector.memset(ones_col, 1.0)
    amat_sb = consts.tile([NT, N_TRIPLETS], F32)
    nc.sync.dma_start(out=amat_sb, in_=amat[:])

    # threshold column: thr = |ct| + 0.1 broadcast to [128,1]
    ct_ap = ct[:]
    ct_bcast = bass.AP(tensor=ct_ap.tensor, offset=ct_ap.offset,
                       ap=[[0, 128]] + list(ct_ap.ap))
    ct_col = consts.tile([128, 1], F32)
    nc.sync.dma_start(out=ct_col, in_=ct_bcast)
    thr_col = consts.tile([128, 1], F32)
    nc.scalar.activation(out=thr_col, in_=ct_col, func=AF.Abs)
    nc.vector.tensor_scalar_add(out=thr_col, in0=thr_col, scalar1=0.1)

    # ---- accumulators ----
    conn_acc = acc.tile([128, B_CORE], F32)
    trace_acc = acc.tile([128, B_CORE], F32)
    edges_all = acc.tile([NT, B_CORE], F32)

    for b in range(B_CORE):
        xtile = xpool.tile([128, NCHUNK, 128], BF16, tag="x")
        nc.sync.dma_start(out=xtile, in_=xt[b])

        gps = gpsum.tile([128, 128], F32, tag="g")
        for c in range(NCHUNK):
            nc.tensor.matmul(gps, lhsT=xtile[:, c, :], rhs=xtile[:, c, :],
                             start=(c == 0), stop=(c == NCHUNK - 1))

        # sqn_col = diag(G)
        sqn_col = sqnpool.tile([128, 1], F32, tag="sqn")
        junk = work.tile([128, 128], BF16, tag="junk")
        nc.vector.scalar_tensor_tensor(
            out=junk, in0=gps, scalar=1.0, in1=ident_f32,
            op0=OP.mult, op1=OP.mult, accum_out=sqn_col)

        # dsq = diag(sqn) in bf16; psum += (-0.5*ones).T @ dsq  => -0.5*sqn_j
        # (on DVE so the matmul's RAW on dsq and WAR on gps share one sem)
        dsq = work.tile([128, 128], BF16, tag="dsq")
        nc.vector.tensor_scalar_mul(out=dsq, in0=ident_bf, scalar1=sqn_col)
        nc.tensor.matmul(gps, lhsT=neghalf, rhs=dsq, start=False, stop=True,
                         skip_group_check=True)

        # r = relu(-2*psum + sqn_i) = relu(||xi - xj||^2)
        rtile = work.tile([128, 128], F32, tag="r")
        nc.scalar.activation(out=rtile, in_=gps, func=AF.Relu,
                             bias=sqn_col, scale=-2.0)
        # d = sqrt(r) in bf16
        dtile = work.tile([128, 128], BF16, tag="d")
        nc.scalar.activation(out=dtile, in_=rtile, func=AF.Sqrt)
        # p = sigmoid(thr - d); accumulate row sums
        pjunk = work.tile([128, 128], F32, tag="p")
        nc.scalar.activation(out=pjunk, in_=dtile, func=AF.Sigmoid,
                             bias=thr_col, scale=-1.0,
                             accum_out=conn_acc[:, b:b + 1])
        # trace of p
        junk2 = work.tile([128, 128], BF16, tag="junk2")
        nc.vector.scalar_tensor_tensor(
            out=junk2, in0=pjunk, scalar=1.0, in1=ident_f32,
            op0=OP.mult, op1=OP.mult, accum_out=trace_acc[:, b:b + 1])

        # triplet gather: O[t, j] = d[r_t, j]
        ohtile = ohpool.tile([128, NT], BF16, tag="oh")
        nc.sync.dma_start(out=ohtile, in_=oh[b])
        cmtile = ohpool.tile([NT, 128], BF16, tag="cm")
        nc.sync.dma_start(out=cmtile, in_=cm[b])
        # reuse the finished gps psum tile for the gather output: both the
        # RAW on dtile and the WAR on gps (via relu) are ACT-sem deps
        ops = gps[:NT, :]
        nc.tensor.matmul(ops, lhsT=ohtile, rhs=dtile, start=True, stop=True,
                         skip_group_check=True)
        junk96 = work.tile([NT, 128], BF16, tag="junk96")
        nc.vector.scalar_tensor_tensor(
            out=junk96, in0=ops, scalar=1.0, in1=cmtile,
            op0=OP.mult, op1=OP.mult, accum_out=edges_all[:, b:b + 1])

    # ---- tail ----
    edges2 = acc.tile([NT, B_CORE], F32)
    nc.vector.tensor_mul(edges2, edges_all, edges_all)
    s1 = spsum.tile([N_TRIPLETS, B_CORE], F32, tag="s1")
    nc.tensor.matmul(s1, lhsT=amat_sb, rhs=edges_all, start=True, stop=True)
    s2 = spsum.tile([N_TRIPLETS, B_CORE], F32, tag="s2")
    nc.tensor.matmul(s2, lhsT=amat_sb, rhs=edges2, start=True, stop=True)
    s1_sb = acc.tile([N_TRIPLETS, B_CORE], F32)
    nc.scalar.copy(out=s1_sb, in_=s1)
    v1 = acc.tile([N_TRIPLETS, B_CORE], F32)
    nc.vector.scalar_tensor_tensor(
        out=v1, in0=s1, scalar=1.0 / 6.0, in1=s1_sb, op0=OP.mult, op1=OP.mult)
    v2 = acc.tile([N_TRIPLETS, B_CORE], F32)
    nc.vector.scalar_tensor_tensor(
        out=v2, in0=s2, scalar=0.5, in1=v1, op0=OP.mult, op1=OP.subtract)
    hole_col = acc.tile([N_TRIPLETS, 1], F32)
    hjunk = acc.tile([N_TRIPLETS, B_CORE], F32)
    nc.scalar.activation(out=hjunk, in_=v2, func=AF.Exp, scale=-1.0,
                         accum_out=hole_col)

    # conn partial: sum(conn_acc) - sum(trace_acc)
    c1 = acc.tile([128, 1], F32)
    nc.vector.reduce_sum(out=c1, in_=conn_acc, axis=mybir.AxisListType.X)
    c2 = acc.tile([128, 1], F32)
    nc.vector.reduce_sum(out=c2, in_=trace_acc, axis=mybir.AxisListType.X)
    c3 = acc.tile([128, 1], F32)
    nc.vector.tensor_sub(c3, c1, c2)

    fin = spsum.tile([1, 2], F32, tag="fin")
    nc.tensor.matmul(fin[:, 0:1], lhsT=c3, rhs=ones_col, start=True, stop=True)
    nc.tensor.matmul(fin[:, 1:2], lhsT=hole_col, rhs=ones_col[:N_TRIPLETS, :],
                     start=True, stop=True, skip_group_check=True)

    outsb = acc.tile([1, 2], F32)
    nc.scalar.copy(out=outsb, in_=fin)
    nc.sync.dma_start(out=out[:], in_=outsb)


_NC_CACHE = None


def build_nc():
    global _NC_CACHE
    if _NC_CACHE is not None:
        return _NC_CACHE
    nc = bacc.Bacc()
    xt = nc.declare_dram_parameter("xt", [B_CORE, 128, NCHUNK, 128], BF16,
                                   isOutput=False)
    oh = nc.declare_dram_parameter("oh", [B_CORE, 128, NT], BF16,
                                   isOutput=False)
    cm = nc.declare_dram_parameter("cm", [B_CORE, NT, 128], BF16,
                                   isOutput=False)
    amat = nc.declare_dram_parameter("amat", [NT, N_TRIPLETS], F32,
                                     isOutput=False)
    ct = nc.declare_dram_parameter("ct", [1], F32, isOutput=False)
    out = nc.declare_dram_parameter("out", [1, 2], F32, isOutput=True)
    with tile.TileContext(nc) as tc, ExitStack() as ctx:
        _build_kernel_body(ctx, tc, xt, oh, cm, amat, ct, out)
    nc.finalize()
    _NC_CACHE = nc
    return nc


def make_in_maps(latent_batch, connection_threshold, triplet_idx):
    latent_batch = np.asarray(latent_batch)
    connection_threshold = np.asarray(connection_threshold, dtype=np.float32)
    triplet_idx = np.asarray(triplet_idx)

    B, T, Dd = latent_batch.shape
    stride = max(T // TC, 1)
    xs = np.ascontiguousarray(latent_batch[:, ::stride, :], dtype=np.float32)
    # [b, i, dim] -> [b, d_local, chunk, i]
    xt_all = np.ascontiguousarray(xs.transpose(0, 2, 1)) \
        .reshape(B, NCHUNK, 128, TC).transpose(0, 2, 1, 3)
    xt_all = np.ascontiguousarray(xt_all).astype(ml_dtypes.bfloat16)

    ti = triplet_idx.astype(np.int64)
    # edge order t = e*32 + k: e0=(i0,i1), e1=(i0,i2), e2=(i1,i2)
    rr = np.concatenate([ti[:, :, 0], ti[:, :, 0], ti[:, :, 1]], axis=1)
    cc = np.concatenate([ti[:, :, 1], ti[:, :, 2], ti[:, :, 2]], axis=1)
    jj = np.arange(TC)
    oh_all = (jj[None, :, None] == rr[:, None, :]).astype(ml_dtypes.bfloat16)
    cm_all = (jj[None, None, :] == cc[:, :, None]).astype(ml_dtypes.bfloat16)

    amat = (np.arange(NT)[:, None] % N_TRIPLETS ==
            np.arange(N_TRIPLETS)[None, :]).astype(np.float32)

    in_maps = []
    for k in range(N_CORES):
        s = slice(k * B_CORE, (k + 1) * B_CORE)
        in_maps.append({
            "xt": xt_all[s],
            "oh": oh_all[s],
            "cm": cm_all[s],
            "amat": amat,
            "ct": connection_threshold,
        })
    return in_maps


def combine_outputs(results):
    s_conn = 0.0
    s_hole = 0.0
    for r in results:
        o = np.asarray(r["out"], dtype=np.float64)
        s_conn += o[0, 0]
        s_hole += o[0, 1]
    conn_mean = 1.0 - s_conn / (B_TOTAL * DENOM)
    hole_mean = s_hole / (B_TOTAL * N_TRIPLETS)
    return np.float32(conn_mean + 0.5 * hole_mean)


def run_cores(latent_batch, connection_threshold, triplet_idx, **kwargs):
    nc = build_nc()
    in_maps = make_in_maps(latent_batch, connection_threshold, triplet_idx)
    return run_bass_kernel_spmd(nc, in_maps, core_ids=list(range(N_CORES)),
                                **kwargs)


def kernel(latent_batch, connection_threshold, triplet_idx):
    res = run_cores(latent_batch, connection_threshold, triplet_idx)
    return combine_outputs(res.results)


if __name__ == "__main__":
    rng = np.random.default_rng(0)
    latent = rng.standard_normal((B_TOTAL, 2048, D), dtype=np.float32)
    ctv = np.ones((1,), dtype=np.float32)
    tri = rng.integers(0, TC, size=(B_TOTAL, N_TRIPLETS, 3), dtype=np.int32)
    print(kernel(latent, ctv, tri))
